# revision 14
# baseline (speedup 1.0000x reference)
"""CVTGAD loss kernel for 8 TRN2 NeuronCores.

Math (matches the jax reference):
  l_node[b] = mean_i [ lse_j(sim_ij) - sim_ii ]   per graph (128x128 InfoNCE)
  l_graph   = InfoNCE over pooled graph embeddings (512x512)
  out = (std(l_node)+1e-6) * mean(l_node) + (std(l_graph)+1e-6) * mean(l_graph)

Sharding: 64 graphs (8192 node rows) per core; h_s_final replicated (rolled per
core so each core's own graphs sit at columns 0:64, making the SPMD diag
core-independent). Device computes per-graph node-loss columns [128,64] and
l_graph [64]; host does the tiny std/mean/weighted-sum epilogue.

Kernel strategy per core (v2 -- PE-offloaded reductions, S^T orientation):
  - SWDGE DMA-cast loads f32->bf16 (dest-byte charged).
  - hs transposed RAW; its row sumsq = DVE square of the transposed tile
    (2x mode) + per-graph ones-matmuls on PE -> PSUM [j,1] -- exactly the
    per-partition layout the Exp scale AP needs (invs_j).
  - hf sumsq in natural layout (ACT Square+accum / DVE ttr split for engine
    balance); 2/tau/|hf_i| folded into hf data via tensor_scalar (4x mode)
    BEFORE its transpose, so the batched Gram needs no per-row scale on the
    i side.
  - Per-graph Grams S^T[j,i] -> PSUM; per-graph ACT Exp with scale=invs_j,
    NO accum_out (saves 187ns/instr); rowsum_i = ones-matmul over the exp
    tile's j-partitions (N=1 matmuls are ~free on PE).
  - pos_i (the diagonal sim_ii) = elementwise product of the two transposed
    tiles + ones-matmul; avoids all identity-mask work.
  - All activations pinned to the natural_log_exp_and_others table set
    (single ACT_TABLE_LOAD; the default picker thrashes ~2.7us reloads).
"""

import numpy as np

B = 512
NPER = 128
D = 256
NCORES = 8
GPC = B // NCORES      # 64 graphs per core
BLK = 8                # graphs per DMA block
NBLK = GPC // BLK
QG = 4                 # graphs per PSUM group (gram/exp granularity)
TAU = 0.5
LN_INV_TAU = float(np.log(1.0 / TAU))

_CACHE = {}


def _build():
    import os
    import concourse.bacc as bacc
    import concourse.tile as tile
    import concourse.mybir as mybir
    import concourse.hw_specs as hw_specs
    from concourse._compat import get_trn_type

    # Pin every activation to the one table set that has Exp+Ln+Square+Copy,
    # so the compiler emits a single ACT_TABLE_LOAD instead of thrashing
    # (each reload costs ~2.7us and the default picker alternates sets).
    if not getattr(hw_specs, "_nle_patched", False):
        _orig_tables = hw_specs.get_activation_tables

        def _only_nle(arch):
            t = _orig_tables(arch)
            keep = "natural_log_exp_and_others"
            return {k: (v if k == keep else set()) for k, v in t.items()}

        hw_specs.get_activation_tables = _only_nle
        bacc.get_activation_tables = _only_nle
        hw_specs._nle_patched = True

    f32 = mybir.dt.float32
    bf16 = mybir.dt.bfloat16
    AF = mybir.ActivationFunctionType
    ALU = mybir.AluOpType

    NSQ_ACT = int(os.environ.get("K_NSQ_ACT", "3"))  # hf sumsq graphs on ACT
    RS_MM = os.environ.get("K_RS", "mm") == "mm"    # rowsum via ones-matmul
    SS_MM = os.environ.get("K_SS", "mm") == "mm"    # hs sumsq via sq-transpose+mm
    POS_MM = os.environ.get("K_POS", "mm") == "mm"  # pos via prodT+mm
    # InstTensorTensorReduce aborts the device runtime (works in CoreSim
    # only) -- keep the TT+TSP split, which the cost model prices the same.
    USE_TTR = os.environ.get("K_TTR", "0") == "1"

    nc = bacc.Bacc(get_trn_type() or "TRN2", target_bir_lowering=False, debug=True)

    def rrsum(pool, tag, in0, in1, accum):
        """accum[:, col] = sum_d in0*in1 along free dim (row dot-product)."""
        p = in0.shape[0]
        if USE_TTR:
            scr_t = pool.tile([p, D], bf16, tag=tag)
            nc.vector.tensor_tensor_reduce(
                out=scr_t, in0=in0, in1=in1, scale=1.0, scalar=0.0,
                op0=mybir.AluOpType.mult, op1=mybir.AluOpType.add,
                accum_out=accum,
            )
        else:
            t1 = pool.tile([p, D], bf16, tag=tag + "a")
            nc.vector.tensor_tensor(t1, in0, in1, op=mybir.AluOpType.mult)
            t2 = pool.tile([p, D], bf16, tag=tag + "b")
            nc.vector.tensor_scalar(
                t2, t1, 1.0, 0.0, op0=mybir.AluOpType.mult,
                op1=mybir.AluOpType.add, accum_out=accum,
            )

    hf = nc.declare_dram_parameter("hf", [GPC * NPER, D], f32, isOutput=False)
    hs = nc.declare_dram_parameter("hs", [GPC * NPER, D], f32, isOutput=False)
    hff = nc.declare_dram_parameter("hff", [GPC, D], f32, isOutput=False)
    hsf = nc.declare_dram_parameter("hsf", [B, D], f32, isOutput=False)
    out_node = nc.declare_dram_parameter("out_node", [NPER, GPC], f32, isOutput=True)
    out_graph = nc.declare_dram_parameter("out_graph", [GPC, 1], f32, isOutput=True)

    with tile.TileContext(nc) as tc:
        with (
            tc.tile_pool(name="consts", bufs=1) as consts,
            tc.tile_pool(name="cols", bufs=1) as colsp,
            tc.tile_pool(name="loads", bufs=int(os.environ.get("K_LOADS", "4"))) as loads,
            tc.tile_pool(name="work", bufs=int(os.environ.get("K_WORK", "3"))) as work,
            tc.tile_pool(name="scr", bufs=int(os.environ.get("K_SCR", "2"))) as scr,
            tc.tile_pool(name="rowsum", bufs=1, space="PSUM") as rowsump,
        ):
            ones_c = consts.tile([128, 1], bf16)
            nc.vector.memset(ones_c, 1.0)
            lntau_c = consts.tile([128, 1], f32)
            nc.vector.memset(lntau_c, LN_INV_TAU)

            # per-graph column stats [128, GPC] f32
            ssq_f = colsp.tile([128, GPC], f32)
            invf2_c = colsp.tile([128, GPC], f32)
            invs2_c = colsp.tile([128, GPC], f32)
            pos_c = colsp.tile([128, GPC], f32)
            ln_scr = colsp.tile([128, GPC], f32)
            lns_scr = colsp.tile([128, GPC], f32)
            lsum_c = colsp.tile([128, GPC], f32)
            l_cols = colsp.tile([128, GPC], f32)
            rowsum_sb = colsp.tile([128, GPC], f32)

            rowsum_ps = rowsump.tile([128, GPC], f32)

            # ---------------- graph-level loss (own PSUM scope) ----------------
            with (
                tc.tile_pool(name="fin", bufs=1) as fin,
                tc.tile_pool(name="fpsum", bufs=1, space="PSUM") as fpsum,
            ):
                hff_bf = fin.tile([GPC, D], bf16)
                nc.gpsimd.dma_start(out=hff_bf, in_=hff[:, :])
                hsf_bf = fin.tile([128, 4, D], bf16)
                nc.gpsimd.dma_start(
                    out=hsf_bf, in_=hsf[:, :].rearrange("(r p) d -> p r d", p=128)
                )

                ssq_ff = fin.tile([GPC, 1], f32)
                sqf_scr = fin.tile([GPC, D], bf16)
                nc.scalar.activation(sqf_scr, hff_bf, AF.Square, accum_out=ssq_ff)

                ssq_sf = fin.tile([128, 4], f32)
                for r in range(4):
                    rrsum(fin, f"sqs{r}", hsf_bf[:, r, :], hsf_bf[:, r, :],
                          ssq_sf[:, r : r + 1])

                lnf_scr = fin.tile([128, 4], f32)
                invs_f = fin.tile([128, 4], f32)
                nc.scalar.activation(lnf_scr, ssq_sf, AF.Ln)
                nc.scalar.activation(invs_f, lnf_scr, AF.Exp, scale=-0.5)

                lnf2_scr = fin.tile([GPC, 1], f32)
                invf2_f = fin.tile([GPC, 1], f32)
                nc.scalar.activation(lnf2_scr, ssq_ff, AF.Ln)
                nc.scalar.activation(
                    invf2_f, lnf2_scr, AF.Exp, scale=-0.5, bias=lntau_c[:GPC]
                )

                hsN_f = fin.tile([128, 4, D], bf16)
                for r in range(4):
                    nc.vector.tensor_scalar_mul(
                        hsN_f[:, r, :], hsf_bf[:, r, :], invs_f[:, r : r + 1]
                    )

                # pos_g = hff_g . hsfN_g (own graphs are rows 0:64 after roll)
                pos_f = fin.tile([GPC, 1], f32)
                rrsum(fin, "posf", hff_bf, hsN_f[:GPC, 0, :], pos_f)

                hffT = fin.tile([128, 2, GPC], bf16)
                nc.sync.dma_start(out=hffT, in_=hff_bf, transpose=True)
                hsfT = fin.tile([128, 4, 2, 128], bf16)
                nc.sync.dma_start(out=hsfT, in_=hsN_f, transpose=True)

                sfin_ps = fpsum.tile([GPC, 512], f32)
                for c in range(2):
                    nc.tensor.matmul(
                        sfin_ps, hffT[:, c, :], hsfT[:, :, c, :],
                        start=(c == 0), stop=(c == 1),
                    )

                expf = fin.tile([GPC, 512], bf16)
                rowsum_f = fin.tile([GPC, 1], f32)
                nc.scalar.activation(
                    expf, sfin_ps, AF.Exp, scale=invf2_f, accum_out=rowsum_f
                )

                lnr_f = fin.tile([GPC, 1], f32)
                nc.scalar.activation(lnr_f, rowsum_f, AF.Ln)
                posx_f = fin.tile([GPC, 1], f32)
                nc.vector.tensor_tensor(posx_f, pos_f, invf2_f, op=ALU.mult)
                lg = fin.tile([GPC, 1], f32)
                nc.vector.tensor_tensor(lg, lnr_f, posx_f, op=ALU.subtract)
                nc.sync.dma_start(out=out_graph[:, :], in_=lg)

            # ---------------- node-level loss ----------------
            hf_r = hf[:, :].rearrange("(g p) d -> p g d", p=128)
            hs_r = hs[:, :].rearrange("(g p) d -> p g d", p=128)
            with (
                tc.tile_pool(name="spsum", bufs=int(os.environ.get("K_SPSUM", "2")), space="PSUM") as spsum,
                tc.tile_pool(name="statps", bufs=int(os.environ.get("K_STATPS", "2")), space="PSUM") as statps,
            ):
                for b in range(NBLK):
                    bs = slice(b * BLK, (b + 1) * BLK)
                    hf_bf = loads.tile([128, BLK, D], bf16, tag="hf_bf")
                    nc.gpsimd.dma_start(out=hf_bf, in_=hf_r[:, bs, :])
                    hs_bf = loads.tile([128, BLK, D], bf16, tag="hs_bf")
                    nc.gpsimd.dma_start(out=hs_bf, in_=hs_r[:, bs, :])

                    # raw hs transpose: only depends on the load, keeps the
                    # DMA engines busy while hf-side stats compute.
                    tT_s = work.tile([128, BLK, 2, 128], bf16, tag="tT_s")
                    nc.sync.dma_start(out=tT_s, in_=hs_bf, transpose=True)

                    # hf row sumsq (ACT/DVE split for engine balance)
                    for g in range(BLK):
                        gg = b * BLK + g
                        if g < NSQ_ACT:
                            sq_scr = scr.tile([128, D], bf16, tag="sqa")
                            nc.scalar.activation(
                                sq_scr, hf_bf[:, g, :], AF.Square,
                                accum_out=ssq_f[:, gg : gg + 1],
                            )
                        else:
                            rrsum(scr, "sqv", hf_bf[:, g, :], hf_bf[:, g, :],
                                  ssq_f[:, gg : gg + 1])
                    nc.scalar.activation(ln_scr[:, bs], ssq_f[:, bs], AF.Ln)
                    nc.scalar.activation(
                        invf2_c[:, bs], ln_scr[:, bs], AF.Exp,
                        scale=-0.5, bias=lntau_c,
                    )

                    # fold 2/tau/|hf_i| into hf, then transpose
                    hfN = work.tile([128, BLK, D], bf16, tag="hfN")
                    for g in range(BLK):
                        gg = b * BLK + g
                        nc.vector.tensor_scalar_mul(
                            hfN[:, g, :], hf_bf[:, g, :], invf2_c[:, gg : gg + 1]
                        )
                    tT_f = work.tile([128, BLK, 2, 128], bf16, tag="tT_f")
                    nc.sync.dma_start(out=tT_f, in_=hfN, transpose=True)

                    # hs row sumsq: square transposed tile + ones-matmuls
                    stat_ps = statps.tile([128, 2, BLK], f32, tag="stat")
                    if SS_MM:
                        sq_sT = work.tile([128, BLK, 2, 128], bf16, tag="sq_sT")
                        nc.vector.tensor_tensor(sq_sT, tT_s, tT_s, op=ALU.mult)
                        for g in range(BLK):
                            for c in range(2):
                                nc.tensor.matmul(
                                    stat_ps[:, 0, g : g + 1],
                                    sq_sT[:, g, c, :], ones_c,
                                    start=(c == 0), stop=(c == 1),
                                )
                        nc.scalar.activation(lns_scr[:, bs], stat_ps[:, 0, :], AF.Ln)
                    else:
                        ssq_s_sb = lsum_c  # borrow as scratch cols
                        for g in range(BLK):
                            gg = b * BLK + g
                            rrsum(scr, "sqsv", hs_bf[:, g, :], hs_bf[:, g, :],
                                  ssq_s_sb[:, gg : gg + 1])
                        nc.scalar.activation(lns_scr[:, bs], ssq_s_sb[:, bs], AF.Ln)
                    nc.scalar.activation(
                        invs2_c[:, bs], lns_scr[:, bs], AF.Exp, scale=-0.5
                    )

                    # pos_i = sum_d hfN*hs (then * invs_j at the end of block)
                    if POS_MM:
                        prodT = work.tile([128, BLK, 2, 128], bf16, tag="prodT")
                        nc.vector.tensor_tensor(prodT, tT_f, tT_s, op=ALU.mult)
                        for g in range(BLK):
                            for c in range(2):
                                nc.tensor.matmul(
                                    stat_ps[:, 1, g : g + 1],
                                    prodT[:, g, c, :], ones_c,
                                    start=(c == 0), stop=(c == 1),
                                )
                        nc.vector.tensor_tensor(
                            pos_c[:, bs], stat_ps[:, 1, :], invs2_c[:, bs], op=ALU.mult
                        )
                    else:
                        posr_sb = ln_scr  # borrow as scratch cols
                        for g in range(BLK):
                            gg = b * BLK + g
                            rrsum(scr, "posm", hfN[:, g, :], hs_bf[:, g, :],
                                  posr_sb[:, gg : gg + 1])
                        nc.vector.tensor_tensor(
                            pos_c[:, bs], posr_sb[:, bs], invs2_c[:, bs], op=ALU.mult
                        )

                    # grams (S^T), per-graph exp (scale=invs_j), rowsum matmul
                    for q in range(BLK // QG):
                        s_ps = spsum.tile([128, QG, 128], f32, tag="s_ps")
                        eT = work.tile([128, QG, 128], bf16, tag="eT")
                        for j in range(QG):
                            g = q * QG + j
                            gg = b * BLK + g
                            for c in range(2):
                                nc.tensor.matmul(
                                    s_ps[:, j, :],
                                    tT_s[:, g, c, :],
                                    tT_f[:, g, c, :],
                                    start=(c == 0), stop=(c == 1),
                                )
                            if RS_MM:
                                nc.scalar.activation(
                                    eT[:, j, :], s_ps[:, j, :], AF.Exp,
                                    scale=invs2_c[:, gg : gg + 1],
                                )
                                nc.tensor.matmul(
                                    rowsum_ps[:, gg : gg + 1], eT[:, j, :], ones_c,
                                    start=True, stop=True,
                                )
                            else:
                                nc.scalar.activation(
                                    eT[:, j, :], s_ps[:, j, :], AF.Exp,
                                    scale=invs2_c[:, gg : gg + 1],
                                    accum_out=rowsum_sb[:, gg : gg + 1],
                                )

                if RS_MM:
                    nc.scalar.activation(lsum_c, rowsum_ps, AF.Ln)
                else:
                    nc.scalar.activation(lsum_c, rowsum_sb, AF.Ln)
                nc.vector.tensor_tensor(l_cols, lsum_c, pos_c, op=ALU.subtract)
                nc.sync.dma_start(out=out_node[:, :], in_=l_cols)

    nc.compile()
    return nc


def _get_nc():
    if "nc" not in _CACHE:
        _CACHE["nc"] = _build()
    return _CACHE["nc"]


def _run(in_maps, **kwargs):
    from concourse.bass_utils import run_bass_kernel_spmd

    return run_bass_kernel_spmd(_get_nc(), in_maps, core_ids=list(range(NCORES)), **kwargs)


def make_in_maps(h_f_final, h_s_final, h_f, h_s):
    h_f = np.ascontiguousarray(np.asarray(h_f, dtype=np.float32))
    h_s = np.ascontiguousarray(np.asarray(h_s, dtype=np.float32))
    h_f_final = np.ascontiguousarray(np.asarray(h_f_final, dtype=np.float32))
    h_s_final = np.ascontiguousarray(np.asarray(h_s_final, dtype=np.float32))
    rows = GPC * NPER
    in_maps = []
    for c in range(NCORES):
        in_maps.append(
            {
                "hf": h_f[c * rows : (c + 1) * rows],
                "hs": h_s[c * rows : (c + 1) * rows],
                "hff": h_f_final[c * GPC : (c + 1) * GPC],
                "hsf": np.ascontiguousarray(np.roll(h_s_final, -GPC * c, axis=0)),
            }
        )
    return in_maps


def finish(results):
    l_node = np.concatenate(
        [r["out_node"].astype(np.float64).mean(axis=0) for r in results]
    )
    l_graph = np.concatenate([r["out_graph"][:, 0].astype(np.float64) for r in results])
    lam1 = l_node.std() + 1e-6
    lam2 = l_graph.std() + 1e-6
    return np.float32(lam1 * l_node.mean() + lam2 * l_graph.mean())


def kernel(h_f_final, h_s_final, h_f, h_s, batch=None, **_unused):
    res = _run(make_in_maps(h_f_final, h_s_final, h_f, h_s))
    return finish(res.results)
